# revision 3
# baseline (speedup 1.0000x reference)
"""Expert-parallel MoE MLP (8 experts -> 8 NeuronCores) Bass kernel for TRN2.

Problem: y[t] = W2[e] @ gelu(W1[e] @ x[t] + b1[e]) + b2[e], tokens contiguous
per expert, 2048 tokens/expert, d_in=d_out=1024, d_hid=4096.

Sharding: expert-parallel. Core e gets expert e's weights and its 2048 tokens.
No cross-core communication needed (counts are equal and tokens are already
sorted by expert); host does the shard/unshard.

Per-core compute layout (everything partition-major, h kept as [hid, tok]):
  GEMM1: h[hid, tok]  = w1T[k,:].T @ xT[k, tok]   (accum over k = d_in tiles)
  GELU : h = gelu(h + b1)  via ScalarE with fused per-partition bias
  GEMM2: y[dout, tok] = w2T[k,:].T @ h[k, tok]    (accum over k = d_hid tiles)
  BIAS : y += b2 via ScalarE Identity with fused bias (fp16 out -> half DMA)

Head-latency optimizations (v2): prologue DMA issue is split across the two
HWDGE trigger engines (Sync + Scalar) because each DMA_DIRECT2D issue costs
~0.65us on its engine and the baseline's single-queue serial issue delayed
the first real matmul to ~16.5us. First x k-piece + first w1 piece issue
immediately on separate engines, warmup is trimmed to 8 matmuls (just enough
to bridge the framework prologue to first-data), and a dummy gelu forces the
ScalarE activation-table load before the first real gelu needs it.
"""
import sys

sys.path.insert(0, "/opt/trn_rl_repo")

import numpy as np

import concourse.bass as bass  # noqa: F401
import concourse.tile as tile
from concourse import bacc, mybir
from concourse.bass_utils import run_bass_kernel_spmd

E = 8
T_PER_E = 2048
D_IN = 1024
D_HID = 4096
D_OUT = 1024

TOK_BLK = 512          # tokens per block (= PSUM bank free size in fp32)
N_TOK_BLK = T_PER_E // TOK_BLK
K1 = D_IN // 128       # k tiles for GEMM1
M1 = D_HID // 128      # output row tiles for GEMM1
K2 = D_HID // 128      # k tiles for GEMM2
M2 = D_OUT // 128      # output row tiles for GEMM2

CDT = mybir.dt.float16   # compute dtype on device (weights + activations)
NP_CDT = np.float16

N_WARM = 8               # warmup matmuls: bridge prologue end -> first data

_compiled = None


def _build():
    nc = bacc.Bacc("TRN2", target_bir_lowering=False, debug=False)

    # Host-permuted layouts (see _make_in_maps):
    #   xL [128, t*4096 + k*512 + c]   = x[t*512+c, k*128+p]
    #   w1L[128, m*1024 + k*128 + mc]  = w1[m*128+mc, k*128+p]
    #   w2L[128, d*4096 + k*128 + dc]  = w2[d*128+dc, k*128+p]
    xL = nc.dram_tensor("xL", [128, N_TOK_BLK * K1 * TOK_BLK], CDT, kind="ExternalInput").ap()
    w1L = nc.dram_tensor("w1L", [128, M1 * K1 * 128], CDT, kind="ExternalInput").ap()
    w2L = nc.dram_tensor("w2L", [128, M2 * K2 * 128], CDT, kind="ExternalInput").ap()
    b1r = nc.dram_tensor("b1r", [128, M1], mybir.dt.float32, kind="ExternalInput").ap()
    b2r = nc.dram_tensor("b2r", [128, M2], mybir.dt.float32, kind="ExternalInput").ap()
    yT = nc.dram_tensor("yT", [D_OUT, T_PER_E], CDT, kind="ExternalOutput").ap()

    XBLK = K1 * TOK_BLK  # 4096 cols per token block in xL
    KW = TOK_BLK         # 512 cols per x k-piece

    with tile.TileContext(nc) as tc:
        with tc.tile_pool(name="wpool", bufs=1) as wpool, \
             tc.tile_pool(name="xpool", bufs=2) as xpool, \
             tc.tile_pool(name="hpool", bufs=1) as hpool, \
             tc.tile_pool(name="opool", bufs=4) as opool, \
             tc.tile_pool(name="ps1", bufs=4, space="PSUM") as ps1, \
             tc.tile_pool(name="ps2", bufs=4, space="PSUM") as ps2:

            # --- PE warmup scratch (memset on GpSimd: its prologue ends
            # earliest and it is otherwise idle) ---
            scr = wpool.tile([128, 128], CDT, name="scr")
            nc.gpsimd.memset(scr[:], 0.0)
            # tiny sbuf tile to trigger the ScalarE gelu table load early
            jnk = wpool.tile([128, 1], mybir.dt.float32, name="jnk")
            nc.gpsimd.memset(jnk[:], 0.0)

            for i in range(N_WARM):
                wps = ps1.tile([128, 128], mybir.dt.float32, tag="ps1", name=f"warm{i}")
                nc.tensor.matmul(wps[:], scr[:], scr[:], start=True, stop=True)

            # === Prologue DMAs, split across three DMA-trigger engines ===
            # Three independent queues (~107 GB/s each): Sync HWDGE carries
            # the w1 m-tile stream, Scalar HWDGE + GpSimd SWDGE split x
            # block 0 (whose last k-piece gates the whole GEMM1 pipeline).
            x_blocks = {}
            x_sb = xpool.tile([128, XBLK], CDT, tag="x", name="x_sb0")
            x_blocks[0] = x_sb
            # Scalar queue: x0 k0..k3, b1 interleaved
            for k in (0, 1):
                nc.scalar.dma_start(x_sb[:, k * KW:(k + 1) * KW],
                                    xL[:, k * KW:(k + 1) * KW])
            b1_sb = wpool.tile([128, M1], mybir.dt.float32, name="b1_sb")
            nc.scalar.dma_start(b1_sb[:], b1r[:, :])
            for k in (2, 3):
                nc.scalar.dma_start(x_sb[:, k * KW:(k + 1) * KW],
                                    xL[:, k * KW:(k + 1) * KW])
            # force the ScalarE activation-table load for Gelu now, while
            # the PE is still chewing on the first m-tiles
            jnk2 = wpool.tile([128, 1], mybir.dt.float32, name="jnk2")
            nc.scalar.activation(jnk2[:], jnk[:],
                                 mybir.ActivationFunctionType.Gelu,
                                 scale=1.0)
            b2_sb = wpool.tile([128, M2], mybir.dt.float32, name="b2_sb")
            nc.scalar.dma_start(b2_sb[:], b2r[:, :])

            # GpSimd SWDGE queue: x0 k4..k7, then whole x block 1
            for k in (4, 5, 6, 7):
                nc.gpsimd.dma_start(x_sb[:, k * KW:(k + 1) * KW],
                                    xL[:, k * KW:(k + 1) * KW])
            x_sb1 = xpool.tile([128, XBLK], CDT, tag="x", name="x_sb1")
            nc.gpsimd.dma_start(x_sb1[:], xL[:, XBLK:2 * XBLK])
            x_blocks[1] = x_sb1

            # Sync queue: w1 m0 in two halves (k0-3, k4-7), then the w1
            # m-tile stream, w2 d-tiles, x blocks 2/3.
            w1_sb = wpool.tile([128, M1 * K1 * 128], CDT, name="w1_sb")
            mw = K1 * 128
            nc.sync.dma_start(w1_sb[:, 0:mw // 2], w1L[:, 0:mw // 2])
            nc.sync.dma_start(w1_sb[:, mw // 2:mw], w1L[:, mw // 2:mw])
            for m in range(1, M1):
                nc.sync.dma_start(w1_sb[:, m * mw:(m + 1) * mw],
                                  w1L[:, m * mw:(m + 1) * mw])

            w2_sb = wpool.tile([128, M2 * K2 * 128], CDT, name="w2_sb")
            dw = K2 * 128
            for d in range(M2):
                nc.sync.dma_start(w2_sb[:, d * dw:(d + 1) * dw],
                                  w2L[:, d * dw:(d + 1) * dw])

            for t in range(N_TOK_BLK):
                if t in x_blocks:
                    x_sb = x_blocks[t]
                else:
                    x_sb = xpool.tile([128, XBLK], CDT, tag="x", name=f"x_sb{t}")
                    nc.sync.dma_start(x_sb[:], xL[:, t * XBLK:(t + 1) * XBLK])

                # --- GEMM1 + gelu: h[m] tiles ---
                h_tiles = []
                for m in range(M1):
                    psum = ps1.tile([128, TOK_BLK], mybir.dt.float32,
                                    tag="ps1", name=f"ps1_{t}_{m}")
                    for k in range(K1):
                        nc.tensor.matmul(
                            psum[:],
                            w1_sb[:, m * (K1 * 128) + k * 128: m * (K1 * 128) + (k + 1) * 128],
                            x_sb[:, k * TOK_BLK:(k + 1) * TOK_BLK],
                            start=(k == 0), stop=(k == K1 - 1),
                        )
                    h_sb = hpool.tile([128, TOK_BLK], CDT, tag=f"h{m}",
                                      name=f"h_sb{t}_{m}")
                    nc.scalar.activation(h_sb[:], psum[:],
                                         mybir.ActivationFunctionType.Gelu,
                                         bias=b1_sb[:, m:m + 1], scale=1.0)
                    h_tiles.append(h_sb)

                # --- GEMM2 + bias: y[d] tiles (fp16 out, DMA off Scalar) ---
                for d in range(M2):
                    psum = ps2.tile([128, TOK_BLK], mybir.dt.float32,
                                    tag="ps2", name=f"ps2_{t}_{d}")
                    for k in range(K2):
                        nc.tensor.matmul(
                            psum[:],
                            w2_sb[:, d * (K2 * 128) + k * 128: d * (K2 * 128) + (k + 1) * 128],
                            h_tiles[k][:],
                            start=(k == 0), stop=(k == K2 - 1),
                        )
                    o_sb = opool.tile([128, TOK_BLK], CDT,
                                      tag="o", name=f"o_sb{t}_{d}")
                    nc.scalar.activation(o_sb[:], psum[:],
                                         mybir.ActivationFunctionType.Identity,
                                         bias=b2_sb[:, d:d + 1], scale=1.0)
                    nc.scalar.dma_start(yT[d * 128:(d + 1) * 128,
                                           t * TOK_BLK:(t + 1) * TOK_BLK],
                                        o_sb[:])

    nc.compile()
    return nc


def _get_compiled():
    global _compiled
    if _compiled is None:
        _compiled = _build()
    return _compiled


def _make_in_maps(x, w1, b1, w2, b2):
    in_maps = []
    for e in range(E):
        xe = x[e * T_PER_E:(e + 1) * T_PER_E]            # [2048, 1024]
        xl = xe.reshape(N_TOK_BLK, TOK_BLK, K1, 128)     # t, c, k, p
        xl = xl.transpose(3, 0, 2, 1).reshape(128, -1)   # p, (t k c)
        w1e = w1[e].reshape(M1, 128, K1, 128)            # m, mc, k, p
        w1l = w1e.transpose(3, 0, 2, 1).reshape(128, -1)  # p, (m k mc)
        w2e = w2[e].reshape(M2, 128, K2, 128)            # d, dc, k, p
        w2l = w2e.transpose(3, 0, 2, 1).reshape(128, -1)  # p, (d k dc)
        in_maps.append({
            "xL": np.ascontiguousarray(xl).astype(NP_CDT),
            "w1L": np.ascontiguousarray(w1l).astype(NP_CDT),
            "w2L": np.ascontiguousarray(w2l).astype(NP_CDT),
            "b1r": np.ascontiguousarray(b1[e].reshape(M1, 128).T).astype(np.float32),
            "b2r": np.ascontiguousarray(b2[e].reshape(M2, 128).T).astype(np.float32),
        })
    return in_maps


def run(x, cnt, w1, b1, w2, b2, trace=False):
    nc = _get_compiled()
    in_maps = _make_in_maps(x, w1, b1, w2, b2)
    res = run_bass_kernel_spmd(nc, in_maps, core_ids=list(range(E)), trace=trace)
    outs = [res.results[e]["yT"].T for e in range(E)]
    y = np.concatenate(outs, axis=0).astype(np.float32)
    return y, res


def kernel(x, cnt, w1, b1, w2, b2):
    y, _ = run(x, cnt, w1, b1, w2, b2, trace=False)
    return y
